# revision 36
# baseline (speedup 1.0000x reference)
"""Multi-head attention on 8 Trainium2 NeuronCores (Bass/Tile, SPMD).

Problem: B=2, S=2048, d_model=128, n_heads=8, per-head dim 128.
    q/k/v = x @ W{q,k,v} + b   -> [B,S,H,128] -> heads
    attn  = softmax(q k^T / sqrt(128))  (mask is per-query-row)
    out   = concat_h(attn @ v) @ Wo + bo

Sharding: 16 (batch, head) pairs over 8 cores -> 2 heads of one batch per
core (data + head parallel).  Each core computes its heads' K-projection,
attention, and the partial output projection sum_{h in core} ctx_h @ Wo_h.
The host sums the 4 partial outputs per batch and adds the bias terms.

Key structure (v2), per core, all seq S = 2048, d = 128:
    Weight folds (host, weight-only transforms):
      Wkq_h = Wk_h @ Wq_h^T   -> scores^T = (key Wkq_h) query^T, so the
              Q-projection disappears; raw query^T is the matmul rhs.
      Wvo_h = Wv_h @ Wo_h     -> AV uses raw value rows; the combined
              projection is applied after attention.
    K~^T_h = Wkq_h^T @ Xk^T   [128, S]  (device projection, f32r)
    Per (head, 1024-wide query pair), per key tile j (16 tiles):
      S^T tile: matmul(lhsT=K~^T[:, j], rhs=Xq^T[:, half]) -> psum [128k, 1024q]
      exp via ScalarE (scale=1/sqrt(d) folded) -> et_j in BF16
      U += Xv_j^T @ et_j  (bf16 matmul, f32 psum accumulate; Wv unapplied)
    Rowsum r = sum_k exp: DVE pre-sums the 16 bf16 et tiles pairwise in a
    3-level tree (14 adds at 2x 16-bit DVE rate) -> 2 sum tiles; then 2
    ones-matmul pairs (vs 16) reduce over partitions into psum [1, 1024].
    outU_h^T = Wvo_h^T @ U;  host: out[b] = sum ot_h^T / r + biases.

    PSUM budget (8 banks): scores triple-buffer 3x[128,1024] (6), ctx/po
    single-buffer pool [128,1024] (2); the transient rowsum [1,1024] tile
    borrows a scores-pool slot.
    Unit tails (rowsum matmuls, evictions, output projection) are deferred
    into the next unit after its first few score tiles, and each unit's
    trailing AV matmuls interleave with the next unit's first score tiles,
    so the PE's in-order queue never blocks on DVE/ACT results.

Bias handling (exact):
    bq: scores term (K bk... ) -> per-key bias column added inside exp when
        bq != 0 (slow path, sbias input); all other bias terms are constant
        per score row and cancel in softmax.
    bv, bo: rows of softmax sum to 1 -> contribute bv @ Wo + bo on the host
        (normalization commutes with the output projection).
    mask: masks whole query rows -> fixed up on the host (all-True here).

Precision: scores/proj matmuls in float32r (TF32-like, 1 cycle/row); AV and
rowsum matmuls in bf16 (same rate; ~0.2% weight rounding, output L2 err
~3e-3 vs the 2e-2 gate).  PSUM accumulation is always f32.
"""

import os

import numpy as np

B = 2
S = 2048
D = 128
H = 8
P = 128
NCORES = 8
HPC = H * B // NCORES  # heads per core = 2
QC = 512               # query positions per matmul
Q2 = 2 * QC            # query positions per unit
NQC = S // QC          # 4
NU = S // Q2           # 2 query units per head
NJ = S // P            # 16 key-position tiles
SCALE = 1.0 / np.sqrt(np.float32(D))

_CACHE = {}


def _build(with_sbias: bool, repeat: int = 1):
    import concourse.bacc as bacc
    import concourse.mybir as mybir
    from concourse.tile import TileContext

    F32 = mybir.dt.float32
    F32R = mybir.dt.float32r
    BF16 = mybir.dt.bfloat16
    PS = mybir.dt.float32
    EXP = mybir.ActivationFunctionType.Exp
    ADD = mybir.AluOpType.add

    nc = bacc.Bacc()
    xq = nc.declare_dram_parameter("xq_t", [P, S], F32R, isOutput=False)
    xk = nc.declare_dram_parameter("xk_t", [P, S], F32R, isOutput=False)
    xv = nc.declare_dram_parameter("xv_n", [P, S], BF16, isOutput=False)
    wkq = nc.declare_dram_parameter("wkq", [P, HPC * P], F32R, isOutput=False)
    wvo = nc.declare_dram_parameter("wvo", [P, HPC * P], F32R, isOutput=False)
    sbias = None
    if with_sbias:
        # (Xk @ Wk @ bq_h) / sqrt(d), [128 partitions, NJ] per head
        sbias = nc.declare_dram_parameter("sbias", [P, HPC * NJ], F32,
                                          isOutput=False)
    # per-head unnormalized projected context (ctxU_h @ Wo_h)^T and the
    # softmax row-sums; the host divides and sums over heads/cores
    out = nc.declare_dram_parameter("out_t", [HPC * P, S], F32, isOutput=True)
    rsum = nc.declare_dram_parameter("rsum", [HPC, S], F32, isOutput=True)

    with TileContext(nc) as tc:
        with (
            tc.tile_pool(name="const", bufs=1) as const,
            tc.tile_pool(name="kt", bufs=2) as ktp,
            tc.tile_pool(name="exps", bufs=7) as exps,
            tc.tile_pool(name="tree1", bufs=3) as tree1,
            tc.tile_pool(name="tree2", bufs=3) as tree2,
            tc.tile_pool(name="tree3", bufs=3) as tree3,
            tc.tile_pool(name="small", bufs=3) as small,
            tc.tile_pool(name="ps_sc", bufs=3, space="PSUM") as ps_sc,
            tc.tile_pool(name="ps_acc", bufs=1, space="PSUM") as ps_acc,
        ):
            # ---- constants ----
            ones = const.tile([P, 1], BF16, tag="ones")
            nc.vector.memset(ones[:], 1.0)

            # ---- load inputs (f32r shares the fp32 memory format) ----
            wkq_r = const.tile([P, HPC * P], F32R, tag="wkq")
            wvo_r = const.tile([P, HPC * P], F32R, tag="wvo")
            nc.sync.dma_start(wkq_r[:], wkq[:])
            nc.sync.dma_start(wvo_r[:], wvo[:])
            xq_r = const.tile([P, S], F32R, tag="xq")
            xk_r = const.tile([P, S], F32R, tag="xk")
            xv_r = const.tile([P, S], BF16, tag="xv")
            # dma_start triggers are costly on the issuing engine; spread
            # them over the three DMA-capable engines (SP/gpsimd/ACT)
            for c in range(NQC):
                sl = slice(c * QC, (c + 1) * QC)
                nc.sync.dma_start(xq_r[:, sl], xq[:, sl])
                nc.gpsimd.dma_start(xk_r[:, sl], xk[:, sl])
                nc.scalar.dma_start(xv_r[:, sl], xv[:, sl])
            sb_t = None
            if with_sbias:
                sb_t = const.tile([P, HPC * NJ], F32, tag="sb")
                nc.sync.dma_start(sb_t[:], sbias[:])

            import contextlib

            # The For_i back-edge waits for every in-flight DMA and resets
            # the DMA semaphores (a cross-engine barrier, ~5-8us).  Unroll
            # several bodies per trip so the barrier amortizes; the deferred
            # tails/projections pipeline across the unrolled seams.
            unroll = 16 if repeat > 1 else 1
            trips = (repeat + unroll - 1) // unroll
            if repeat > 1:
                loop = tc.For_i(0, trips, 1, hint_engines=(
                    mybir.EngineType.PE, mybir.EngineType.Activation,
                    mybir.EngineType.DVE, mybir.EngineType.SP,
                    mybir.EngineType.Pool))
            else:
                loop = contextlib.nullcontext()
            with loop:
                # ---- K~ projection (the only on-device projection) ----
                # kt is double-buffered so projections never WAR-wait on the
                # previous body's reads.  Head 0 of body b+1 projects at the
                # end of body b (overlapping its final tail); head 1 of each
                # body is deferred into that body's unit (0,0) j-loop.  The
                # ScalarE therefore only starves across the For_i back-edge
                # barrier, once per `unroll` bodies.
                def emit_proj_chunk(h, kt_h, c):
                    sl0 = slice(c * QC, (c + 1) * QC)
                    sl1 = slice((c + 1) * QC, (c + 2) * QC)
                    pk = ps_sc.tile([P, Q2], PS, tag="sc", name="pk")
                    nc.tensor.matmul(pk[:, :QC],
                                     wkq_r[:, h * P:(h + 1) * P],
                                     xk_r[:, sl0], start=True, stop=True)
                    nc.tensor.matmul(pk[:, QC:],
                                     wkq_r[:, h * P:(h + 1) * P],
                                     xk_r[:, sl1], start=True, stop=True)
                    nc.vector.tensor_copy(kt_h[:, sl0], pk[:, :QC])
                    nc.vector.tensor_copy(kt_h[:, sl1], pk[:, QC:])

                def emit_proj_h0():
                    kt0 = ktp.tile([P, S], F32R, tag="kt0", name="kt0")
                    for c in range(0, NQC, 2):
                        emit_proj_chunk(0, kt0, c)
                    return kt0

                # unit tails (rowsum matmuls, evictions, output projection,
                # DMAs) are deferred until the NEXT unit has issued a few
                # score tiles, so the PE's in-order stream never waits on
                # DVE/ACT; the last unit's tail flows across the body seam.
                def emit_tail(h, u, accbox, s2s):
                    def tail():
                        acc = accbox[0]
                        # rowsum: ones^T over the 4 tree-summed tiles; the
                        # [1,Q2] psum tile borrows a scores-pool slot, so
                        # evict it promptly (rs_sb first in the DVE queue)
                        rs = ps_sc.tile([1, Q2], PS, tag="sc", name="rs")
                        ng = len(s2s)
                        for g, s2 in enumerate(s2s):
                            st, sp = g == 0, g == ng - 1
                            nc.tensor.matmul(rs[0:1, :QC], ones[:],
                                             s2[:, :QC], start=st, stop=sp)
                            nc.tensor.matmul(rs[0:1, QC:], ones[:],
                                             s2[:, QC:], start=st, stop=sp)
                        rs_sb = small.tile([1, Q2], F32, tag="rs_sb",
                                           name="rs_sb")
                        nc.vector.tensor_copy(rs_sb[:], rs[0:1, :])
                        nc.sync.dma_start(
                            rsum[h:h + 1, u * Q2:(u + 1) * Q2], rs_sb[:])
                        cs = small.tile([P, Q2], F32R, tag="cs", name="cs")
                        nc.vector.tensor_copy(cs[:], acc[:])
                        po = ps_acc.tile([P, Q2], PS, tag="acc", name="po")
                        wh = wvo_r[:, h * P:(h + 1) * P]
                        nc.tensor.matmul(po[:, :QC], wh, cs[:, :QC],
                                         start=True, stop=True)
                        nc.tensor.matmul(po[:, QC:], wh, cs[:, QC:],
                                         start=True, stop=True)
                        ot = small.tile([P, Q2], F32, tag="ot", name="ot")
                        nc.vector.tensor_copy(ot[:], po[:])
                        nc.gpsimd.dma_start(
                            out[h * P:(h + 1) * P, u * Q2:(u + 1) * Q2],
                            ot[:])
                    return tail

                prev_tail = None
                prev_work = None  # (consume_av, trailing pend) of prev unit
                kt0 = emit_proj_h0()
                for body in range(unroll):
                    kt1 = ktp.tile([P, S], F32R, tag="kt1", name="kt1")
                    kt = [kt0, kt1]
                    proj_pending = [(1, kt1, 0), (1, kt1, 2)]
                    for u in range(NU):
                        qa = slice(u * Q2, u * Q2 + QC)
                        qb = slice(u * Q2 + QC, (u + 1) * Q2)
                        for h in range(HPC):
                            accbox = [None]

                            def consume_av(j, et, accbox=accbox):
                                if accbox[0] is None:
                                    accbox[0] = ps_acc.tile(
                                        [P, Q2], PS, tag="acc", name="acc")
                                acc = accbox[0]
                                st, sp = j == 0, j == NJ - 1
                                vj = xv_r[:, j * P:(j + 1) * P]
                                nc.tensor.matmul(acc[:, :QC], vj,
                                                 et[:, :QC],
                                                 start=st, stop=sp)
                                nc.tensor.matmul(acc[:, QC:], vj,
                                                 et[:, QC:],
                                                 start=st, stop=sp)

                            pend = []
                            s1s, s2s, s3s = [], [], []
                            ets = []
                            for j in range(NJ):
                                # previous unit's tail: flushed once three
                                # score tiles are in flight, so its ~2.1us
                                # of rowsum/outproj matmuls run while the
                                # ScalarE drains the exp backlog
                                if j == 3 and prev_tail is not None:
                                    prev_tail()
                                    prev_tail = None
                                kj = kt[h][:, j * P:(j + 1) * P]
                                sc = ps_sc.tile([P, Q2], PS, tag="sc",
                                                name="sc")
                                nc.tensor.matmul(sc[:, :QC], kj,
                                                 xq_r[:, qa],
                                                 start=True, stop=True)
                                nc.tensor.matmul(sc[:, QC:], kj,
                                                 xq_r[:, qb],
                                                 start=True, stop=True)
                                et = exps.tile([P, Q2], BF16, tag="exp",
                                               name="et")
                                if with_sbias:
                                    bias = sb_t[:, h * NJ + j:
                                                h * NJ + j + 1]
                                    nc.scalar.activation(et[:, :QC],
                                                         sc[:, :QC],
                                                         EXP, bias=bias,
                                                         scale=float(SCALE))
                                    nc.scalar.activation(et[:, QC:],
                                                         sc[:, QC:],
                                                         EXP, bias=bias,
                                                         scale=float(SCALE))
                                else:
                                    nc.scalar.activation(et[:], sc[:], EXP,
                                                         scale=float(SCALE))
                                ets.append(et)
                                # 3-level pairwise tree on DVE (bf16, 2x
                                # 16-bit rate): 16 et tiles -> 2 sum tiles,
                                # so the PE rowsum shrinks to 4 matmuls
                                if j % 2 == 1:
                                    s1 = tree1.tile([P, Q2], BF16,
                                                    tag="s1", name="s1")
                                    nc.vector.tensor_tensor(
                                        s1[:], ets[j - 1][:], et[:], op=ADD)
                                    s1s.append(s1)
                                    if len(s1s) == 2:
                                        s2 = tree2.tile([P, Q2], BF16,
                                                        tag="s2", name="s2")
                                        nc.vector.tensor_tensor(
                                            s2[:], s1s[0][:], s1s[1][:],
                                            op=ADD)
                                        s2s.append(s2)
                                        s1s = []
                                        if len(s2s) % 2 == 0:
                                            s3 = tree3.tile(
                                                [P, Q2], BF16,
                                                tag="s3", name="s3")
                                            nc.vector.tensor_tensor(
                                                s3[:], s2s[-2][:],
                                                s2s[-1][:], op=ADD)
                                            s2s[-2:] = []
                                            s3s.append(s3)
                                pend.append((j, et))
                                # previous unit's trailing AVs, interleaved
                                # with our first score tiles (each AV has
                                # its own stationary, so this costs no
                                # extra weight loads and keeps ACT fed)
                                if j <= 2 and prev_work is not None:
                                    pfn, ppend = prev_work
                                    if ppend:
                                        pfn(*ppend.pop(0))
                                    if not ppend:
                                        prev_work = None
                                if len(pend) > 3:
                                    consume_av(*pend.pop(0))
                                # deferred head-1 projection, overlapped
                                # with unit (0,0)'s attention stream
                                if j in (6, 10) and proj_pending:
                                    emit_proj_chunk(*proj_pending.pop(0))
                            prev_work = (consume_av, pend)
                            prev_tail = emit_tail(h, u, accbox, s3s)
                    if body + 1 < unroll:
                        # next body's head-0 projection, overlapping the
                        # final tail of this body
                        kt0 = emit_proj_h0()
                if prev_work is not None:
                    pfn, ppend = prev_work
                    for pr in ppend:
                        pfn(*pr)
                if prev_tail is not None:
                    prev_tail()

    nc.compile()
    return nc


def _get_nc(with_sbias: bool):
    key = ("nc", with_sbias)
    if key not in _CACHE:
        _CACHE[key] = _build(with_sbias)
    return _CACHE[key]


def kernel(query, key, value, mask, Wq, bq, Wk, bk, Wv, bv, Wo, bo):
    import ml_dtypes
    from concourse.bass_utils import run_bass_kernel_spmd

    query = np.asarray(query, np.float32)
    key_ = np.asarray(key, np.float32)
    value = np.asarray(value, np.float32)
    mask = np.asarray(mask, bool)
    Wq, Wk, Wv, Wo = (np.asarray(a, np.float32) for a in (Wq, Wk, Wv, Wo))
    bq, bk, bv, bo = (np.asarray(a, np.float32) for a in (bq, bk, bv, bo))

    with_sbias = bool(np.any(bq != 0))
    nc = _get_nc(with_sbias)

    in_maps = []
    for c in range(NCORES):
        b = c // (NCORES // B)
        h0 = (c % (NCORES // B)) * HPC
        m = {
            "xq_t": np.ascontiguousarray(query[b].T),
            "xk_t": np.ascontiguousarray(key_[b].T),
            # value in natural row-chunks: block j = rows j*128..+128
            "xv_n": np.ascontiguousarray(
                value[b].reshape(NJ, P, P).transpose(1, 0, 2).reshape(P, S)
            ).astype(ml_dtypes.bfloat16),
            # per-head weight folds (weight-only transforms)
            "wkq": np.ascontiguousarray(np.concatenate(
                [Wk[:, (h0 + h) * P:(h0 + h + 1) * P]
                 @ Wq[:, (h0 + h) * P:(h0 + h + 1) * P].T
                 for h in range(HPC)], axis=1)),
            "wvo": np.ascontiguousarray(np.concatenate(
                [Wv[:, (h0 + h) * P:(h0 + h + 1) * P]
                 @ Wo[(h0 + h) * P:(h0 + h + 1) * P, :]
                 for h in range(HPC)], axis=1)),
        }
        if with_sbias:
            sb = np.zeros((P, HPC * NJ), np.float32)
            for h in range(HPC):
                col = Wk[:, (h0 + h) * P:(h0 + h + 1) * P] @ bq[(h0 + h) * P:
                                                               (h0 + h + 1) * P]
                v = (key_[b] @ col) * SCALE  # [S]
                sb[:, h * NJ:(h + 1) * NJ] = v.reshape(NJ, P).T
            m["sbias"] = sb
        in_maps.append(m)

    trace = os.environ.get("ATTN_TRACE") == "1"
    res = run_bass_kernel_spmd(nc, in_maps, list(range(NCORES)), trace=trace)
    _CACHE["last_result"] = res

    out = np.zeros((B, S, P), np.float32)
    for c in range(NCORES):
        b = c // (NCORES // B)
        ot = np.asarray(res.results[c]["out_t"])   # [HPC*P, S]
        rs = np.asarray(res.results[c]["rsum"])    # [HPC, S]
        for h in range(HPC):
            out[b] += ot[h * P:(h + 1) * P].T / rs[h][:, None]
    out += (bo + bv @ Wo)[None, None, :]

    if not mask.all():
        # masked query rows see a uniform distribution over all keys
        for b in range(B):
            bad = ~mask[b]
            if bad.any():
                ctx_u = value[b].mean(axis=0) @ Wv + bv  # [H*P]
                out[b, bad, :] = ctx_u @ Wo + bo
    return out.astype(np.float32)


# revision 37
# speedup vs baseline: 1.9560x; 1.9560x over previous
"""Multi-head attention on 8 Trainium2 NeuronCores (Bass/Tile, SPMD).

Problem: B=2, S=2048, d_model=128, n_heads=8, per-head dim 128.
    q/k/v = x @ W{q,k,v} + b   -> [B,S,H,128] -> heads
    attn  = softmax(q k^T / sqrt(128))  (mask is per-query-row)
    out   = concat_h(attn @ v) @ Wo + bo

Sharding: 16 (batch, head) pairs over 8 cores -> 2 heads of one batch per
core (data + head parallel).  Each core computes its heads' K-projection,
attention, and the partial output projection sum_{h in core} ctx_h @ Wo_h.
The host sums the 4 partial outputs per batch and adds the bias terms.

Key structure (v2), per core, all seq S = 2048, d = 128:
    Weight folds (host, weight-only transforms):
      Wkq_h = Wk_h @ Wq_h^T   -> scores^T = (key Wkq_h) query^T, so the
              Q-projection disappears; raw query^T is the matmul rhs.
      Wvo_h = Wv_h @ Wo_h     -> AV uses raw value rows; the combined
              projection is applied after attention.
    K~^T_h = Wkq_h^T @ Xk^T   [128, S]  (device projection, f32r)
    Per (head, 1024-wide query pair), per key tile j (16 tiles):
      S^T tile: matmul(lhsT=K~^T[:, j], rhs=Xq^T[:, half]) -> psum [128k, 1024q]
      exp via ScalarE (scale=1/sqrt(d) folded) -> et_j in BF16
      U += Xv_j^T @ et_j  (bf16 matmul, f32 psum accumulate; Wv unapplied)
    Rowsum r = sum_k exp: DVE pre-sums the 16 bf16 et tiles pairwise in a
    3-level tree (14 adds at 2x 16-bit DVE rate) -> 2 sum tiles; then 2
    ones-matmul pairs (vs 16) reduce over partitions into psum [1, 1024].
    outU_h^T = Wvo_h^T @ U;  host: out[b] = sum ot_h^T / r + biases.

    PSUM budget (8 banks): scores triple-buffer 3x[128,1024] (6), ctx/po
    single-buffer pool [128,1024] (2); the transient rowsum [1,1024] tile
    borrows a scores-pool slot.
    Unit tails (rowsum matmuls, evictions, output projection) are deferred
    into the next unit after its first few score tiles, and each unit's
    trailing AV matmuls interleave with the next unit's first score tiles,
    so the PE's in-order queue never blocks on DVE/ACT results.

Bias handling (exact):
    bq: scores term (K bk... ) -> per-key bias column added inside exp when
        bq != 0 (slow path, sbias input); all other bias terms are constant
        per score row and cancel in softmax.
    bv, bo: rows of softmax sum to 1 -> contribute bv @ Wo + bo on the host
        (normalization commutes with the output projection).
    mask: masks whole query rows -> fixed up on the host (all-True here).

Precision: scores/proj matmuls in float32r (TF32-like, 1 cycle/row); AV and
rowsum matmuls in bf16 (same rate; ~0.2% weight rounding, output L2 err
~3e-3 vs the 2e-2 gate).  PSUM accumulation is always f32.
"""

import os

import numpy as np

B = 2
S = 2048
D = 128
H = 8
P = 128
NCORES = 8
HPC = H * B // NCORES  # heads per core = 2
QC = 512               # query positions per matmul
Q2 = 2 * QC            # query positions per unit
NQC = S // QC          # 4
NU = S // Q2           # 2 query units per head
NJ = S // P            # 16 key-position tiles
SCALE = 1.0 / np.sqrt(np.float32(D))

_CACHE = {}


def _build(with_sbias: bool, repeat: int = 1):
    import concourse.bacc as bacc
    import concourse.mybir as mybir
    from concourse.tile import TileContext

    F32 = mybir.dt.float32
    F32R = mybir.dt.float32r
    BF16 = mybir.dt.bfloat16
    PS = mybir.dt.float32
    EXP = mybir.ActivationFunctionType.Exp
    ADD = mybir.AluOpType.add

    nc = bacc.Bacc()
    xq = nc.declare_dram_parameter("xq_t", [P, S], F32R, isOutput=False)
    xk = nc.declare_dram_parameter("xk_t", [P, S], F32R, isOutput=False)
    xv = nc.declare_dram_parameter("xv_n", [P, S], BF16, isOutput=False)
    wkq = nc.declare_dram_parameter("wkq", [P, HPC * P], F32R, isOutput=False)
    wvo = nc.declare_dram_parameter("wvo", [P, HPC * P], F32R, isOutput=False)
    sbias = None
    if with_sbias:
        # (Xk @ Wk @ bq_h) / sqrt(d), [128 partitions, NJ] per head
        sbias = nc.declare_dram_parameter("sbias", [P, HPC * NJ], F32,
                                          isOutput=False)
    # per-head unnormalized projected context (ctxU_h @ Wo_h)^T and the
    # softmax row-sums; the host divides and sums over heads/cores
    out = nc.declare_dram_parameter("out_t", [HPC * P, S], F32, isOutput=True)
    rsum = nc.declare_dram_parameter("rsum", [HPC, S], F32, isOutput=True)

    with TileContext(nc) as tc:
        with (
            tc.tile_pool(name="const", bufs=1) as const,
            tc.tile_pool(name="kt", bufs=2) as ktp,
            tc.tile_pool(name="exps", bufs=7) as exps,
            tc.tile_pool(name="tree1", bufs=3) as tree1,
            tc.tile_pool(name="tree2", bufs=3) as tree2,
            tc.tile_pool(name="tree3", bufs=3) as tree3,
            tc.tile_pool(name="small", bufs=3) as small,
            tc.tile_pool(name="ps_sc", bufs=3, space="PSUM") as ps_sc,
            tc.tile_pool(name="ps_acc", bufs=1, space="PSUM") as ps_acc,
        ):
            # ---- constants ----
            ones = const.tile([P, 1], BF16, tag="ones")
            nc.vector.memset(ones[:], 1.0)

            # ---- load inputs (f32r shares the fp32 memory format) ----
            wkq_r = const.tile([P, HPC * P], F32R, tag="wkq")
            wvo_r = const.tile([P, HPC * P], F32R, tag="wvo")
            nc.sync.dma_start(wkq_r[:], wkq[:])
            nc.sync.dma_start(wvo_r[:], wvo[:])
            xq_r = const.tile([P, S], F32R, tag="xq")
            xk_r = const.tile([P, S], F32R, tag="xk")
            xv_r = const.tile([P, S], BF16, tag="xv")
            # dma_start triggers are costly on the issuing engine; spread
            # them over the three DMA-capable engines (SP/gpsimd/ACT)
            for c in range(NQC):
                sl = slice(c * QC, (c + 1) * QC)
                nc.sync.dma_start(xq_r[:, sl], xq[:, sl])
                nc.gpsimd.dma_start(xk_r[:, sl], xk[:, sl])
                nc.scalar.dma_start(xv_r[:, sl], xv[:, sl])
            sb_t = None
            if with_sbias:
                sb_t = const.tile([P, HPC * NJ], F32, tag="sb")
                nc.sync.dma_start(sb_t[:], sbias[:])

            import contextlib

            # The For_i back-edge waits for every in-flight DMA and resets
            # the DMA semaphores (a cross-engine barrier, ~5-8us).  Unroll
            # several bodies per trip so the barrier amortizes; the deferred
            # tails/projections pipeline across the unrolled seams.
            unroll = 8 if repeat > 1 else 1
            trips = (repeat + unroll - 1) // unroll
            if repeat > 1:
                loop = tc.For_i(0, trips, 1, hint_engines=(
                    mybir.EngineType.PE, mybir.EngineType.Activation,
                    mybir.EngineType.DVE, mybir.EngineType.SP,
                    mybir.EngineType.Pool))
            else:
                loop = contextlib.nullcontext()
            with loop:
                # ---- K~ projection (the only on-device projection) ----
                # kt is double-buffered so projections never WAR-wait on the
                # previous body's reads.  Head 0 of body b+1 projects at the
                # end of body b (overlapping its final tail); head 1 of each
                # body is deferred into that body's unit (0,0) j-loop.  The
                # ScalarE therefore only starves across the For_i back-edge
                # barrier, once per `unroll` bodies.
                def emit_proj_chunk(h, kt_h, c):
                    sl0 = slice(c * QC, (c + 1) * QC)
                    sl1 = slice((c + 1) * QC, (c + 2) * QC)
                    pk = ps_sc.tile([P, Q2], PS, tag="sc", name="pk")
                    nc.tensor.matmul(pk[:, :QC],
                                     wkq_r[:, h * P:(h + 1) * P],
                                     xk_r[:, sl0], start=True, stop=True)
                    nc.tensor.matmul(pk[:, QC:],
                                     wkq_r[:, h * P:(h + 1) * P],
                                     xk_r[:, sl1], start=True, stop=True)
                    nc.vector.tensor_copy(kt_h[:, sl0], pk[:, :QC])
                    nc.vector.tensor_copy(kt_h[:, sl1], pk[:, QC:])

                def emit_proj_h0():
                    kt0 = ktp.tile([P, S], F32R, tag="kt0", name="kt0")
                    for c in range(0, NQC, 2):
                        emit_proj_chunk(0, kt0, c)
                    return kt0

                # unit tails (rowsum matmuls, evictions, output projection,
                # DMAs) are deferred until the NEXT unit has issued a few
                # score tiles, so the PE's in-order stream never waits on
                # DVE/ACT; the last unit's tail flows across the body seam.
                def emit_tail(h, u, accbox, s2s):
                    def tail():
                        acc = accbox[0]
                        # rowsum: ones^T over the 4 tree-summed tiles; the
                        # [1,Q2] psum tile borrows a scores-pool slot, so
                        # evict it promptly (rs_sb first in the DVE queue)
                        rs = ps_sc.tile([1, Q2], PS, tag="sc", name="rs")
                        ng = len(s2s)
                        for g, s2 in enumerate(s2s):
                            st, sp = g == 0, g == ng - 1
                            nc.tensor.matmul(rs[0:1, :QC], ones[:],
                                             s2[:, :QC], start=st, stop=sp)
                            nc.tensor.matmul(rs[0:1, QC:], ones[:],
                                             s2[:, QC:], start=st, stop=sp)
                        rs_sb = small.tile([1, Q2], F32, tag="rs_sb",
                                           name="rs_sb")
                        nc.vector.tensor_copy(rs_sb[:], rs[0:1, :])
                        nc.sync.dma_start(
                            rsum[h:h + 1, u * Q2:(u + 1) * Q2], rs_sb[:])
                        cs = small.tile([P, Q2], F32R, tag="cs", name="cs")
                        nc.vector.tensor_copy(cs[:], acc[:])
                        po = ps_acc.tile([P, Q2], PS, tag="acc", name="po")
                        wh = wvo_r[:, h * P:(h + 1) * P]
                        nc.tensor.matmul(po[:, :QC], wh, cs[:, :QC],
                                         start=True, stop=True)
                        nc.tensor.matmul(po[:, QC:], wh, cs[:, QC:],
                                         start=True, stop=True)
                        ot = small.tile([P, Q2], F32, tag="ot", name="ot")
                        nc.vector.tensor_copy(ot[:], po[:])
                        nc.gpsimd.dma_start(
                            out[h * P:(h + 1) * P, u * Q2:(u + 1) * Q2],
                            ot[:])
                    return tail

                prev_tail = None
                prev_work = None  # (consume_av, trailing pend) of prev unit
                kt0 = emit_proj_h0()
                for body in range(unroll):
                    kt1 = ktp.tile([P, S], F32R, tag="kt1", name="kt1")
                    kt = [kt0, kt1]
                    proj_pending = [(1, kt1, 0), (1, kt1, 2)]
                    for u in range(NU):
                        qa = slice(u * Q2, u * Q2 + QC)
                        qb = slice(u * Q2 + QC, (u + 1) * Q2)
                        for h in range(HPC):
                            accbox = [None]

                            def consume_av(j, et, accbox=accbox):
                                if accbox[0] is None:
                                    accbox[0] = ps_acc.tile(
                                        [P, Q2], PS, tag="acc", name="acc")
                                acc = accbox[0]
                                st, sp = j == 0, j == NJ - 1
                                vj = xv_r[:, j * P:(j + 1) * P]
                                nc.tensor.matmul(acc[:, :QC], vj,
                                                 et[:, :QC],
                                                 start=st, stop=sp)
                                nc.tensor.matmul(acc[:, QC:], vj,
                                                 et[:, QC:],
                                                 start=st, stop=sp)

                            pend = []
                            s1s, s2s, s3s = [], [], []
                            ets = []
                            for j in range(NJ):
                                # previous unit's tail: flushed once three
                                # score tiles are in flight, so its ~2.1us
                                # of rowsum/outproj matmuls run while the
                                # ScalarE drains the exp backlog
                                if j == 3 and prev_tail is not None:
                                    prev_tail()
                                    prev_tail = None
                                kj = kt[h][:, j * P:(j + 1) * P]
                                sc = ps_sc.tile([P, Q2], PS, tag="sc",
                                                name="sc")
                                nc.tensor.matmul(sc[:, :QC], kj,
                                                 xq_r[:, qa],
                                                 start=True, stop=True)
                                nc.tensor.matmul(sc[:, QC:], kj,
                                                 xq_r[:, qb],
                                                 start=True, stop=True)
                                et = exps.tile([P, Q2], BF16, tag="exp",
                                               name="et")
                                if with_sbias:
                                    bias = sb_t[:, h * NJ + j:
                                                h * NJ + j + 1]
                                    nc.scalar.activation(et[:, :QC],
                                                         sc[:, :QC],
                                                         EXP, bias=bias,
                                                         scale=float(SCALE))
                                    nc.scalar.activation(et[:, QC:],
                                                         sc[:, QC:],
                                                         EXP, bias=bias,
                                                         scale=float(SCALE))
                                else:
                                    nc.scalar.activation(et[:], sc[:], EXP,
                                                         scale=float(SCALE))
                                ets.append(et)
                                # 3-level pairwise tree on DVE (bf16, 2x
                                # 16-bit rate): 16 et tiles -> 2 sum tiles,
                                # so the PE rowsum shrinks to 4 matmuls
                                if j % 2 == 1:
                                    s1 = tree1.tile([P, Q2], BF16,
                                                    tag="s1", name="s1")
                                    nc.vector.tensor_tensor(
                                        s1[:], ets[j - 1][:], et[:], op=ADD)
                                    s1s.append(s1)
                                    if len(s1s) == 2:
                                        s2 = tree2.tile([P, Q2], BF16,
                                                        tag="s2", name="s2")
                                        nc.vector.tensor_tensor(
                                            s2[:], s1s[0][:], s1s[1][:],
                                            op=ADD)
                                        s2s.append(s2)
                                        s1s = []
                                        if len(s2s) % 2 == 0:
                                            s3 = tree3.tile(
                                                [P, Q2], BF16,
                                                tag="s3", name="s3")
                                            nc.vector.tensor_tensor(
                                                s3[:], s2s[-2][:],
                                                s2s[-1][:], op=ADD)
                                            s2s[-2:] = []
                                            s3s.append(s3)
                                pend.append((j, et))
                                # previous unit's trailing AVs, interleaved
                                # with our first score tiles (each AV has
                                # its own stationary, so this costs no
                                # extra weight loads and keeps ACT fed)
                                if j <= 2 and prev_work is not None:
                                    pfn, ppend = prev_work
                                    if ppend:
                                        pfn(*ppend.pop(0))
                                    if not ppend:
                                        prev_work = None
                                if len(pend) > 3:
                                    consume_av(*pend.pop(0))
                                # deferred head-1 projection, overlapped
                                # with unit (0,0)'s attention stream
                                if j in (6, 10) and proj_pending:
                                    emit_proj_chunk(*proj_pending.pop(0))
                            prev_work = (consume_av, pend)
                            prev_tail = emit_tail(h, u, accbox, s3s)
                    if body + 1 < unroll:
                        # next body's head-0 projection, overlapping the
                        # final tail of this body
                        kt0 = emit_proj_h0()
                if prev_work is not None:
                    pfn, ppend = prev_work
                    for pr in ppend:
                        pfn(*pr)
                if prev_tail is not None:
                    prev_tail()

    nc.compile()
    return nc


def _get_nc(with_sbias: bool):
    key = ("nc", with_sbias)
    if key not in _CACHE:
        _CACHE[key] = _build(with_sbias)
    return _CACHE[key]


def kernel(query, key, value, mask, Wq, bq, Wk, bk, Wv, bv, Wo, bo):
    import ml_dtypes
    from concourse.bass_utils import run_bass_kernel_spmd

    query = np.asarray(query, np.float32)
    key_ = np.asarray(key, np.float32)
    value = np.asarray(value, np.float32)
    mask = np.asarray(mask, bool)
    Wq, Wk, Wv, Wo = (np.asarray(a, np.float32) for a in (Wq, Wk, Wv, Wo))
    bq, bk, bv, bo = (np.asarray(a, np.float32) for a in (bq, bk, bv, bo))

    with_sbias = bool(np.any(bq != 0))
    nc = _get_nc(with_sbias)

    in_maps = []
    for c in range(NCORES):
        b = c // (NCORES // B)
        h0 = (c % (NCORES // B)) * HPC
        m = {
            "xq_t": np.ascontiguousarray(query[b].T),
            "xk_t": np.ascontiguousarray(key_[b].T),
            # value in natural row-chunks: block j = rows j*128..+128
            "xv_n": np.ascontiguousarray(
                value[b].reshape(NJ, P, P).transpose(1, 0, 2).reshape(P, S)
            ).astype(ml_dtypes.bfloat16),
            # per-head weight folds (weight-only transforms)
            "wkq": np.ascontiguousarray(np.concatenate(
                [Wk[:, (h0 + h) * P:(h0 + h + 1) * P]
                 @ Wq[:, (h0 + h) * P:(h0 + h + 1) * P].T
                 for h in range(HPC)], axis=1)),
            "wvo": np.ascontiguousarray(np.concatenate(
                [Wv[:, (h0 + h) * P:(h0 + h + 1) * P]
                 @ Wo[(h0 + h) * P:(h0 + h + 1) * P, :]
                 for h in range(HPC)], axis=1)),
        }
        if with_sbias:
            sb = np.zeros((P, HPC * NJ), np.float32)
            for h in range(HPC):
                col = Wk[:, (h0 + h) * P:(h0 + h + 1) * P] @ bq[(h0 + h) * P:
                                                               (h0 + h + 1) * P]
                v = (key_[b] @ col) * SCALE  # [S]
                sb[:, h * NJ:(h + 1) * NJ] = v.reshape(NJ, P).T
            m["sbias"] = sb
        in_maps.append(m)

    trace = os.environ.get("ATTN_TRACE") == "1"
    res = run_bass_kernel_spmd(nc, in_maps, list(range(NCORES)), trace=trace)
    _CACHE["last_result"] = res

    out = np.zeros((B, S, P), np.float32)
    for c in range(NCORES):
        b = c // (NCORES // B)
        ot = np.asarray(res.results[c]["out_t"])   # [HPC*P, S]
        rs = np.asarray(res.results[c]["rsum"])    # [HPC, S]
        for h in range(HPC):
            out[b] += ot[h * P:(h + 1) * P].T / rs[h][:, None]
    out += (bo + bv @ Wo)[None, None, :]

    if not mask.all():
        # masked query rows see a uniform distribution over all keys
        for b in range(B):
            bad = ~mask[b]
            if bad.any():
                ctx_u = value[b].mean(axis=0) @ Wv + bv  # [H*P]
                out[b, bad, :] = ctx_u @ Wo + bo
    return out.astype(np.float32)
